# revision 17
# baseline (speedup 1.0000x reference)
"""Trainium2 Bass kernel for nn_Attention_23759759081800.

Fused attention block: qkv proj + QK-LayerNorm + LPE (per-channel affine on v)
+ softmax attention + output proj, for x (2, 2048, 1024), 16 heads, d=64.

Sharding over 8 NeuronCores: data-parallel over batch (2 groups of 4 cores)
x head-parallel (4 heads per core).

Key structure (v2, rebuilt for PE-bound execution):
- x / w_qkv / q,k features in bf16 (1 cycle/row on PE, 2x-4x DVE elementwise,
  half DMA); v / attention-prob / residual path in fp32r.
- LayerNorm stats (sum, sumsq over the full 1024 channels) are reduced with
  one small AllReduce PER 512-token chunk so the stats -> rstd -> normalize
  pipeline overlaps the remaining qkv matmuls.  Per-token rows (rstd, mu*rstd)
  are replicated across partitions with DMA row-broadcasts (DRAM-sourced,
  stride-0 partition dim) instead of PE rank-1 matmuls; the per-channel
  gamma/beta affine rides a 4x-mode DVE tensor_scalar with [P,1] operands.
- v is computed once (feature-major, with the LPE affine folded into the
  PSUM->resid copy) and transposed to key-major layout on the PE (identity
  matmul transpose), instead of a second set of matmuls.
- Attention: per head-pair, per 128-key tile: scores (bf16), exp, AV (fp32r)
  with the row-sum folded in via an appended ones-column on V (ones column
  is LAST for even local heads and FIRST for odd heads so the two AV outputs
  land on disjoint PSUM partitions with aligned bases).  The exp is split
  across engines: the even-head half always uses the Scalar engine's exact
  Exp; the odd-head half uses a Schraudolph fast-exp (one fused
  tensor_scalar, float->int trick, |rel err| < 3%) on the Vector engine for
  middle key-tiles and on GpSimd for the first/last quarter, so no single
  engine paces the loop - the PE does.
- The softmax 1/rowsum epilogue is software-pipelined one iteration behind
  (reciprocal -> ones-broadcast matmul -> multiply -> resid add on GpSimd).
- Output projection is split into 2 waves (first/second half of tokens),
  each with its own 8-rank AllToAll, so wave 0's collective + proj overlap
  the second half of attention.
"""
import sys

if "/opt/trn_rl_repo" not in sys.path:
    sys.path.insert(0, "/opt/trn_rl_repo")

import numpy as np
import ml_dtypes
import concourse.bass as bass
import concourse.mybir as mybir
import concourse.tile as tile
from concourse import bacc
from concourse.bass import ts
from concourse.bass_utils import run_bass_kernel_spmd
from concourse.masks import make_identity

F32 = mybir.dt.float32
F32R = mybir.dt.float32r
BF16 = mybir.dt.bfloat16
I32 = mybir.dt.int32
AF = mybir.ActivationFunctionType
ALU = mybir.AluOpType

B, C, H, D = 2, 1024, 16, 64
N_TOK_FULL = 2048
P = 128
LOCF = 256  # channels per core
GROUPS = [[0, 1, 2, 3], [4, 5, 6, 7]]

# Schraudolph fast-exp: exp(x) ~= bitcast_f32(int32(A*x + B)).  B tuned for
# truncating float->int conversion; max rel err 3.0% on [-6, 6].
EXP_A = float(1 << 23) / float(np.log(2.0))
EXP_B = float(127 << 23) - 368000.0


def round_fp32r(x: np.ndarray) -> np.ndarray:
    """Round fp32 to fp32r (e8m11: top 20 bits), round-to-nearest-even-ish."""
    v = np.ascontiguousarray(x, dtype=np.float32).view(np.uint32)
    r = v + 0x7FF + ((v >> 12) & 1)
    r &= np.uint32(0xFFFFF000)
    return r.view(np.float32)


def to_bf16(x: np.ndarray) -> np.ndarray:
    return np.ascontiguousarray(np.asarray(x, np.float32).astype(ml_dtypes.bfloat16))


def build_nc(n_tok: int = N_TOK_FULL, collectives: bool = True):
    """Build the SPMD program (identical on all 8 cores)."""
    NCH = n_tok // 512   # 512-token chunks
    KT = n_tok // 128    # 128-token key tiles
    TOKT = n_tok // 128  # vaug token tiles
    WTOK = n_tok // 16   # tokens per core per batch per a2a wave
    QCW = max(NCH // 2, 1)  # q-chunks per proj wave

    nc = bacc.Bacc("TRN2", target_bir_lowering=False, debug=False, num_devices=8)

    xt_e = nc.dram_tensor("xt", [C, n_tok], BF16, kind="ExternalInput")
    wt_e = nc.dram_tensor("wt", [6, P, 8, P], BF16, kind="ExternalInput")
    wpt_e = nc.dram_tensor("wpt", [C, C], BF16, kind="ExternalInput")
    gcol_e = nc.dram_tensor("gcol", [P, 8], F32, kind="ExternalInput")
    lpec_e = nc.dram_tensor("lpec", [P, 4], F32, kind="ExternalInput")
    biasb_e = nc.dram_tensor("biasb", [P, C], F32, kind="ExternalInput")
    y_e = nc.dram_tensor("y", [2, 2, WTOK, C], F32, kind="ExternalOutput")

    xt_ap = xt_e.ap().rearrange("(o p) t -> p o t", p=P)       # [128, 8, n_tok]
    wpt_ap = wpt_e.ap().rearrange("(o p) c -> p o c", p=P)     # [128, 8, 1024]

    with tile.TileContext(nc) as tc:
        with (
            tc.tile_pool(name="const", bufs=1) as cpool,
            tc.tile_pool(name="persist", bufs=1) as ppool,
            tc.tile_pool(name="work", bufs=2) as wpool,
            tc.tile_pool(name="et", bufs=3) as etpool,
            tc.tile_pool(name="dram", bufs=1, space="DRAM") as dram,
        ):
            # ---- constants / persistent tiles ----
            wt_sb = ppool.tile([P, 6, 8, P], BF16)
            for f in range(6):
                nc.scalar.dma_start(wt_sb[:, f], wt_e.ap()[f])
            xall = ppool.tile([P, NCH, 8, 512], BF16)
            for c in range(NCH):
                nc.sync.dma_start(xall[:, c], xt_ap[:, :, ts(c, 512)])

            gcols = cpool.tile([P, 8], F32)
            lpecs = cpool.tile([P, 4], F32)
            with tc.high_priority(offset=-1000):
                nc.scalar.dma_start(gcols[:], gcol_e[:])
                nc.scalar.dma_start(lpecs[:], lpec_e[:])
            scol = cpool.tile([P, 1], BF16)
            nc.any.memset(scol[:], 1.0 / C)
            ones1 = cpool.tile([1, P], F32R)
            nc.any.memset(ones1[:].bitcast(F32), 1.0)
            ident = cpool.tile([P, P], F32R)
            make_identity(nc, ident[:].bitcast(F32))

            qf = ppool.tile([P, 2, n_tok], BF16)   # q feature-major [f, fo, tok]
            kf = ppool.tile([P, 2, n_tok], BF16)
            vaug = ppool.tile([P, TOKT, 4, D + 1], F32R)  # [tok, to, head, d|1]
            resid = ppool.tile([P, 2, n_tok], BF16)       # v_lpe + attn out
            nc.gpsimd.memset(vaug[:, :, :, D : D + 1].bitcast(F32), 1.0)

            # proj weights / bias tiles (loaded after P1-B: see below)
            wpt_sb = ppool.tile([P, 8, C], BF16)
            biasb_sb = ppool.tile([P, C], F32)

            # stats scratch (per chunk)
            st_in = [dram.tile([4, 512], BF16, name=f"stin{c}") for c in range(NCH)]
            st_out = [dram.tile([4, 512], BF16, name=f"stout{c}") for c in range(NCH)]
            rows_d = [dram.tile([2, 1, 2, 512], BF16, name=f"rows{c}") for c in range(NCH)]
            a2a_in = [dram.tile([8, LOCF, WTOK], BF16, name=f"a2ai{g}") for g in range(2)]
            a2a_out = [dram.tile([8, LOCF, WTOK], BF16, name=f"a2ao{g}") for g in range(2)]

            rbs = {}  # (chunk, qk2) -> broadcast tile [P, 2(rs|mrs), 512]

            def emit_rows(c):
                """Stats post-processing for chunk c: load reduced stats,
                derive rstd & mu*rstd rows, ship to DRAM, broadcast."""
                mu2 = wpool.tile([1, 2, 512], BF16, tag="mu2", bufs=1)
                s22 = wpool.tile([1, 2, 512], BF16, tag="s22", bufs=1)
                stv = st_out[c].rearrange("(q k) t -> k q t", k=2)
                nc.sync.dma_start(mu2[:], stv[0])   # rows (mu_q, mu_k)
                nc.sync.dma_start(s22[:], stv[1])   # rows (s2_q, s2_k)
                t2 = wpool.tile([1, 2, 512], BF16, tag="t2", bufs=1)
                nc.vector.tensor_tensor(t2[:], mu2[:], mu2[:], ALU.mult)
                nc.vector.tensor_tensor(t2[:], s22[:], t2[:], ALU.subtract)
                t2e = wpool.tile([1, 2, 512], BF16, tag="t2e", bufs=1)
                nc.vector.tensor_scalar(t2e[:], t2[:], 1.0, 1e-5, ALU.mult, ALU.add)
                rv = wpool.tile([1, 2, 512], BF16, tag="rv", bufs=1)
                with nc.allow_low_precision(reason="LN rstd in bf16"):
                    nc.vector.reciprocal(rv[:], t2e[:])
                rs2 = wpool.tile([1, 2, 512], BF16, tag="rs2", bufs=1)
                nc.scalar.activation(rs2[:], rv[:], AF.Sqrt)
                mrs2 = wpool.tile([1, 2, 512], BF16, tag="mrs2", bufs=1)
                nc.vector.tensor_tensor(mrs2[:], mu2[:], rs2[:], ALU.mult)
                if c == NCH - 1:
                    # flip the Act table to the exp set before attention
                    warm = wpool.tile([1, 16], F32R, tag="warm", bufs=1)
                    nc.scalar.activation(warm[:], mrs2[:, 0, 0:16], AF.Exp)
                rdv = rows_d[c]  # [2(qk), 1, 2(kind), 512]
                nc.scalar.dma_start(rdv[:, 0, 0, :], rs2[:])
                nc.scalar.dma_start(rdv[:, 0, 1, :], mrs2[:])
                for qk2 in range(2):
                    rb = wpool.tile([P, 2, 512], BF16, tag="rb", bufs=4)
                    nc.scalar.dma_start(rb[:], rdv[qk2].to_broadcast([P, 2, 512]))
                    rbs[(c, qk2)] = rb

            def emit_norm(c):
                """Apply LN to q,k for chunk c (k first: attention needs k of
                every chunk before q-chunk 0's last key tile)."""
                for qk2, feat in ((1, kf), (0, qf)):
                    rb = rbs[(c, qk2)]
                    t1 = wpool.tile([P, 2, 512], BF16, tag="t1")
                    nc.vector.tensor_tensor(
                        t1[:], feat[:, :, ts(c, 512)],
                        rb[:, 0:1, :].to_broadcast([P, 2, 512]), ALU.mult,
                    )
                    nc.vector.tensor_tensor(
                        t1[:], t1[:],
                        rb[:, 1:2, :].to_broadcast([P, 2, 512]), ALU.subtract,
                    )
                    for fo in range(2):
                        nc.vector.tensor_scalar(
                            feat[:, fo, ts(c, 512)], t1[:, fo, :],
                            gcols[:, 4 * qk2 + fo : 4 * qk2 + fo + 1],
                            gcols[:, 4 * qk2 + 2 + fo : 4 * qk2 + 3 + fo],
                            ALU.mult, ALU.add,
                        )

            # ============ P1-A: q,k projection + stats (per chunk) ============
            with tc.tile_pool(name="ps", bufs=1, space="PSUM") as psA2:
                psP1 = psA2
                for c in range(NCH):
                    for fo4 in range(4):
                        ps = psP1.tile([P, 512], F32, tag="av", bufs=2)
                        for co in range(8):
                            nc.tensor.matmul(
                                ps[:], wt_sb[:, fo4, co, :], xall[:, c, co, :],
                                start=(co == 0), stop=(co == 7),
                            )
                        dest = qf if fo4 < 2 else kf
                        nc.scalar.copy(dest[:, fo4 % 2, ts(c, 512)], ps[:])
                    sqq = wpool.tile([P, 2, 512], BF16, tag="sq")
                    nc.vector.tensor_tensor(
                        sqq[:], qf[:, :, ts(c, 512)], qf[:, :, ts(c, 512)], ALU.mult
                    )
                    sqk = wpool.tile([P, 2, 512], BF16, tag="sq")
                    nc.vector.tensor_tensor(
                        sqk[:], kf[:, :, ts(c, 512)], kf[:, :, ts(c, 512)], ALU.mult
                    )
                    strow = wpool.tile([1, 4, 512], BF16, tag="strow", bufs=1)
                    # order: S1q, S1k (no sq dep) then S2q, S2k
                    stats = [
                        (0, qf[:, :, ts(c, 512)]),
                        (2, kf[:, :, ts(c, 512)]),
                        (1, sqq[:]),
                        (3, sqk[:]),
                    ]
                    for row, src in stats:
                        pst = psP1.tile([1, 512], F32, tag="sc", bufs=6)
                        for fo in range(2):
                            nc.tensor.matmul(
                                pst[:], scol[:], src[:, fo, :],
                                start=(fo == 0), stop=(fo == 1),
                            )
                        nc.gpsimd.tensor_copy(strow[:, row, :], pst[:])
                    nc.sync.dma_start(st_in[c][:], strow[:])
                    if collectives:
                        nc.gpsimd.collective_compute(
                            "AllReduce", ALU.add, replica_groups=GROUPS,
                            ins=[st_in[c].opt()], outs=[st_out[c].opt()],
                        )
                    else:
                        nc.gpsimd.dma_start(st_out[c][:], st_in[c][:])
                    if c >= 1:
                        emit_rows(c - 1)
                    if c >= 2:
                        emit_norm(c - 2)

                # ============ P1-B: v (feature-major) + LPE + transpose ======
                # transposes run one chunk behind the v matmuls so the PE
                # never waits on the PSUM->SBUF vtmp hop; the last chunk's
                # LN application is emitted after its vtmp copies so the DVE
                # queue is not blocked on the (late) chunk-NCH-1 broadcast.
                tasks = [lambda: emit_rows(NCH - 1)]
                tasks += [lambda cc=cc: emit_norm(cc) for cc in range(max(NCH - 2, 0), NCH - 1)]
                def emit_tr(c, vtmps):
                    for tj in range(4):
                        to = 4 * c + tj
                        ptr = psP1.tile([P, 256], F32R, tag="sc", bufs=6)
                        for fo in range(2):
                            nc.tensor.transpose(
                                ptr[:, ts(fo, P)], vtmps[fo][:, ts(tj, P)], ident[:]
                            )
                        pr = ptr[:].rearrange("p (g d) -> p g d", g=4)
                        nc.gpsimd.tensor_copy(vaug[:, to, :, 0:D], pr[:])
                prev_vt = None
                for c in range(NCH):
                    vtmps = []
                    for fo in range(2):
                        psv = psP1.tile([P, 512], F32, tag="av", bufs=2)
                        for co in range(8):
                            nc.tensor.matmul(
                                psv[:], wt_sb[:, 4 + fo, co, :], xall[:, c, co, :],
                                start=(co == 0), stop=(co == 7),
                            )
                        vtmp = wpool.tile([P, 512], F32R, tag="vtmp", bufs=4)
                        nc.vector.tensor_copy(vtmp[:].bitcast(F32), psv[:])
                        nc.scalar.activation(
                            resid[:, fo, ts(c, 512)], psv[:], AF.Identity,
                            scale=lpecs[:, fo : fo + 1], bias=lpecs[:, 2 + fo : 3 + fo],
                        )
                        vtmps.append(vtmp)
                    if prev_vt is not None:
                        emit_tr(c - 1, prev_vt)
                    prev_vt = vtmps
                    if tasks:
                        tasks.pop(0)()
                emit_norm(NCH - 1)
                emit_tr(NCH - 1, prev_vt)
                tasks = []
                with tc.high_priority(offset=-1000000):
                    for co in range(8):
                        nc.gpsimd.dma_start(wpt_sb[:, co, :], wpt_ap[:, co, :])
                    nc.gpsimd.dma_start(biasb_sb[:], biasb_e[:])

                # ============ P6+P8: attention + projection waves ============

                pending = [None]
                pend_avs = []
                pend_rc = [None]

                def emit_recips():
                    if pending[0] is None:
                        return
                    psav0, psav1, qc, fo = pending[0]
                    rc = wpool.tile([1, 2, 512], F32R, tag="rc")
                    with nc.allow_low_precision(reason="softmax 1/sumexp as fp32r"):
                        nc.vector.reciprocal(rc[:, 0, :], psav0[D : D + 1, :])
                        nc.vector.reciprocal(rc[:, 1, :], psav1[D : D + 1, :])
                    pend_rc[0] = rc

                def emit_epilogue():
                    if pending[0] is None:
                        return
                    if pend_rc[0] is None:
                        emit_recips()
                    psav0, psav1, qc, fo = pending[0]
                    pending[0] = None
                    rc = pend_rc[0]
                    pend_rc[0] = None
                    prc0 = psA2.tile([P, 512], F32, tag="sc", bufs=6)
                    prc1 = psA2.tile([P, 512], F32, tag="sc", bufs=6)
                    nc.tensor.matmul(prc0[:], ones1[:], rc[:, 0, :], start=True, stop=True)
                    nc.tensor.matmul(prc1[:], ones1[:], rc[:, 1, :], start=True, stop=True)
                    t_c = wpool.tile([P, 512], F32R, tag="tc")
                    nc.vector.tensor_copy(t_c[64:128, :].bitcast(F32), psav1[0:D, :])
                    t_o = wpool.tile([P, 512], F32R, tag="to")
                    nc.vector.tensor_tensor(
                        t_o[0:D, :], psav0[0:D, :], prc0[0:D, :], ALU.mult
                    )
                    nc.vector.tensor_tensor(
                        t_o[64:128, :], t_c[64:128, :], prc1[64:128, :], ALU.mult
                    )
                    nc.vector.tensor_tensor(
                        resid[0:D, fo, ts(qc, 512)],
                        resid[0:D, fo, ts(qc, 512)], t_o[0:D, :], ALU.add,
                    )
                    nc.vector.tensor_tensor(
                        resid[64:128, fo, ts(qc, 512)],
                        resid[64:128, fo, ts(qc, 512)], t_o[64:128, :], ALU.add,
                    )

                def emit_proj_wave(g):
                    pjb = wpool.tile([P, 2, 8, WTOK], BF16, tag="pjb", bufs=2)
                    for bp in range(2):
                        eng_d = nc.sync if bp == 0 else nc.scalar
                        eng_d.dma_start(
                            pjb[:, bp],
                            a2a_out[g][4 * bp : 4 * bp + 4].rearrange(
                                "r (ci p) t -> p (r ci) t", p=P
                            ),
                        )
                    for bp in range(2):
                        for nch2 in range(2):
                            psy = psA2.tile([P, 512], F32, tag="av", bufs=2)
                            for jc in range(8):
                                nc.tensor.matmul(
                                    psy[0:WTOK, :], pjb[:, bp, jc, :],
                                    wpt_sb[:, jc, ts(nch2, 512)],
                                    start=(jc == 0), stop=(jc == 7),
                                )
                            yt = wpool.tile([P, 512], F32, tag="yt")
                            nc.vector.tensor_tensor(
                                yt[0:WTOK, :], psy[0:WTOK, :],
                                biasb_sb[0:WTOK, ts(nch2, 512)], ALU.add,
                            )
                            eng_y = nc.sync if nch2 == 0 else nc.scalar
                            eng_y.dma_start(
                                y_e[g, bp, :, ts(nch2, 512)], yt[0:WTOK, :]
                            )

                for i in range(2 * NCH):
                    qc, fo = i // 2, i % 2
                    psav0 = psA2.tile([P, 512], F32, tag="av", bufs=2)
                    psav1 = psA2.tile([P, 512], F32, tag="av", bufs=2)
                    ets = {}
                    if pend_avs:
                        pass  # emitted inside the kt loop below
                    for kt in range(KT):
                        et = etpool.tile([P, 2, 512], F32R, tag="et")
                        psc0 = psA2.tile([P, 512], F32, tag="sc", bufs=6)
                        nc.tensor.matmul(
                            psc0[:], kf[0:64, fo, ts(kt, P)],
                            qf[0:64, fo, ts(qc, 512)], start=True, stop=True,
                        )
                        nc.scalar.activation(et[:, 0, :], psc0[:], AF.Exp)
                        psc1 = psA2.tile([P, 512], F32, tag="sc", bufs=6)
                        nc.tensor.matmul(
                            psc1[:], kf[64:128, fo, ts(kt, P)],
                            qf[64:128, fo, ts(qc, 512)], start=True, stop=True,
                        )
                        eng = nc.gpsimd if kt < KT // 2 else nc.vector
                        eng.tensor_scalar(
                            et[:, 1, :].bitcast(I32), psc1[:],
                            EXP_A, EXP_B, ALU.mult, ALU.add,
                        )
                        ets[kt] = et
                        if kt < 2 and pend_avs:
                            pend_avs.pop(0)()
                        if kt == 2:
                            emit_recips()
                        if kt == 4:
                            emit_epilogue()
                        if kt >= 2:
                            _av(nc, vaug, psav0, psav1, fo, kt - 2, ets.pop(kt - 2), KT)
                    for kt in (KT - 2, KT - 1):
                        pend_avs.append(
                            lambda kt=kt, p0=psav0, p1=psav1, f=fo, e=ets.pop(kt): _av(
                                nc, vaug, p0, p1, f, kt, e, KT
                            )
                        )
                    pending[0] = (psav0, psav1, qc, fo)

                    # end of a proj wave's q-chunks: flush epilogue, ship shards
                    if fo == 1 and (qc + 1) % QCW == 0:
                        g = qc // QCW
                        for f_ in pend_avs:
                            f_()
                        pend_avs = []
                        emit_epilogue()
                        base = g * (n_tok // 2)
                        for j in range(8):
                            eng_d = nc.sync if j % 2 == 0 else nc.scalar
                            eng_d.dma_start(
                                a2a_in[g][j].rearrange("(f p) t -> p f t", p=P),
                                resid[:, :, base + j * WTOK : base + (j + 1) * WTOK],
                            )
                        if collectives:
                            nc.gpsimd.collective_compute(
                                "AllToAll", ALU.bypass,
                                replica_groups=[list(range(8))],
                                ins=[a2a_in[g].opt()], outs=[a2a_out[g].opt()],
                            )
                        else:
                            nc.gpsimd.dma_start(a2a_out[g][:], a2a_in[g][:])
                emit_epilogue()
                emit_proj_wave(0)
                pwarm = psA2.tile([P, 512], F32, tag="sc", bufs=6)
                for _ in range(30):
                    nc.tensor.matmul(
                        pwarm[:], kf[0:64, 0, 0:P], qf[0:64, 0, 0:512],
                        start=True, stop=True,
                    )
                emit_proj_wave(1)

    nc.compile()
    return nc


def _av(nc, vaug, psav0, psav1, fo, kt, et, KT):
    """AV accumulation for key tile kt of head pair fo."""
    nc.tensor.matmul(
        psav0[0 : D + 1, :], vaug[:, kt, 2 * fo, :], et[:, 0, :],
        start=(kt == 0), stop=(kt == KT - 1),
    )
    nc.tensor.matmul(
        psav1[0 : D + 1, :], vaug[:, kt, 2 * fo + 1, :], et[:, 1, :],
        start=(kt == 0), stop=(kt == KT - 1),
    )


def prep_in_maps(
    x, w_qkv, q_gamma, q_beta, k_gamma, k_beta, lpe_w, lpe_b, w_proj, b_proj,
    n_tok: int = N_TOK_FULL,
):
    """Shard the full inputs into the 8 per-core input maps."""
    x = np.asarray(x, np.float32)
    w_qkv = np.asarray(w_qkv, np.float32)
    w_proj = np.asarray(w_proj, np.float32)
    vecs = [np.asarray(v, np.float32) for v in
            (q_gamma, q_beta, k_gamma, k_beta, lpe_w, lpe_b, b_proj)]
    q_gamma, q_beta, k_gamma, k_beta, lpe_w, lpe_b, b_proj = vecs

    scale = float(D) ** -0.5
    wq, wk, wv = w_qkv[0:C], w_qkv[C : 2 * C], w_qkv[2 * C : 3 * C]
    wpt = to_bf16(w_proj.T)
    biasb = np.ascontiguousarray(np.broadcast_to(b_proj, (P, C)), np.float32)

    in_maps = []
    for c in range(8):
        b_, hg = c // 4, c % 4
        sl = slice(LOCF * hg, LOCF * hg + LOCF)
        xt = to_bf16(x[b_, :n_tok].T)
        blocks = [wq[sl][0:P], wq[sl][P:LOCF], wk[sl][0:P], wk[sl][P:LOCF],
                  wv[sl][0:P], wv[sl][P:LOCF]]
        wt = np.stack([bl.T.reshape(8, P, P).transpose(1, 0, 2) for bl in blocks])
        wt = to_bf16(wt)

        def two(v):
            return v[sl].reshape(2, P).T  # [p, fo]

        gq = two(q_gamma) * scale
        bq = two(q_beta) * scale
        gk = two(k_gamma)
        bk = two(k_beta)
        gcol = np.concatenate([gq, bq, gk, bk], axis=1).astype(np.float32)  # [128,8]
        lpec = np.concatenate([two(lpe_w), two(lpe_b)], axis=1).astype(np.float32)
        in_maps.append(
            {
                "xt": xt,
                "wt": wt,
                "wpt": wpt,
                "gcol": np.ascontiguousarray(gcol),
                "lpec": np.ascontiguousarray(lpec),
                "biasb": biasb,
            }
        )
    return in_maps


def assemble_y(results, n_tok: int = N_TOK_FULL) -> np.ndarray:
    WTOK = n_tok // 16
    y = np.empty((B, n_tok, C), np.float32)
    for c in range(8):
        yc = np.asarray(results[c]["y"]).reshape(2, 2, WTOK, C)
        for g in range(2):
            lo = g * (n_tok // 2) + c * WTOK
            y[0, lo : lo + WTOK] = yc[g, 0]
            y[1, lo : lo + WTOK] = yc[g, 1]
    return y


_NC_CACHE = {}


def kernel(**inputs) -> np.ndarray:
    key = ("full", N_TOK_FULL)
    if key not in _NC_CACHE:
        _NC_CACHE[key] = build_nc(N_TOK_FULL, collectives=True)
    nc = _NC_CACHE[key]
    in_maps = prep_in_maps(**inputs)
    res = run_bass_kernel_spmd(nc, in_maps, core_ids=list(range(8)))
    return assemble_y(res.results)


# revision 19
# speedup vs baseline: 1.0333x; 1.0333x over previous
"""Trainium2 Bass kernel for nn_Attention_23759759081800.

Fused attention block: qkv proj + QK-LayerNorm + LPE (per-channel affine on v)
+ softmax attention + output proj, for x (2, 2048, 1024), 16 heads, d=64.

Sharding over 8 NeuronCores: data-parallel over batch (2 groups of 4 cores)
x head-parallel (4 heads per core).

Key structure (v2, rebuilt for PE-bound execution):
- x / w_qkv / q,k features in bf16 (1 cycle/row on PE, 2x-4x DVE elementwise,
  half DMA); v / attention-prob / residual path in fp32r.
- LayerNorm stats (sum, sumsq over the full 1024 channels) are reduced with
  one small AllReduce PER 512-token chunk so the stats -> rstd -> normalize
  pipeline overlaps the remaining qkv matmuls.  Per-token rows (rstd, mu*rstd)
  are replicated across partitions with DMA row-broadcasts (DRAM-sourced,
  stride-0 partition dim) instead of PE rank-1 matmuls; the per-channel
  gamma/beta affine rides a 4x-mode DVE tensor_scalar with [P,1] operands.
- v is computed once (feature-major, with the LPE affine folded into the
  PSUM->resid copy) and transposed to key-major layout on the PE (identity
  matmul transpose), instead of a second set of matmuls.
- Attention: per head-pair, per 128-key tile: scores (bf16), exp, AV (fp32r)
  with the row-sum folded in via an appended ones-column on V (ones column
  is LAST for even local heads and FIRST for odd heads so the two AV outputs
  land on disjoint PSUM partitions with aligned bases).  The exp is split
  across engines: the even-head half always uses the Scalar engine's exact
  Exp; the odd-head half uses a Schraudolph fast-exp (one fused
  tensor_scalar, float->int trick, |rel err| < 3%) on the Vector engine for
  middle key-tiles and on GpSimd for the first/last quarter, so no single
  engine paces the loop - the PE does.
- The softmax 1/rowsum epilogue is software-pipelined one iteration behind
  (reciprocal -> ones-broadcast matmul -> multiply -> resid add on GpSimd).
- Output projection is split into 2 waves (first/second half of tokens),
  each with its own 8-rank AllToAll, so wave 0's collective + proj overlap
  the second half of attention.
"""
import sys

if "/opt/trn_rl_repo" not in sys.path:
    sys.path.insert(0, "/opt/trn_rl_repo")

import numpy as np
import ml_dtypes
import concourse.bass as bass
import concourse.mybir as mybir
import concourse.tile as tile
from concourse import bacc
from concourse.bass import ts
from concourse.bass_utils import run_bass_kernel_spmd
from concourse.masks import make_identity

F32 = mybir.dt.float32
F32R = mybir.dt.float32r
BF16 = mybir.dt.bfloat16
I32 = mybir.dt.int32
AF = mybir.ActivationFunctionType
ALU = mybir.AluOpType

B, C, H, D = 2, 1024, 16, 64
N_TOK_FULL = 2048
P = 128
LOCF = 256  # channels per core
GROUPS = [[0, 1, 2, 3], [4, 5, 6, 7]]

# Schraudolph fast-exp: exp(x) ~= bitcast_f32(int32(A*x + B)).  B tuned for
# truncating float->int conversion; max rel err 3.0% on [-6, 6].
EXP_A = float(1 << 23) / float(np.log(2.0))
EXP_B = float(127 << 23) - 368000.0


def round_fp32r(x: np.ndarray) -> np.ndarray:
    """Round fp32 to fp32r (e8m11: top 20 bits), round-to-nearest-even-ish."""
    v = np.ascontiguousarray(x, dtype=np.float32).view(np.uint32)
    r = v + 0x7FF + ((v >> 12) & 1)
    r &= np.uint32(0xFFFFF000)
    return r.view(np.float32)


def to_bf16(x: np.ndarray) -> np.ndarray:
    return np.ascontiguousarray(np.asarray(x, np.float32).astype(ml_dtypes.bfloat16))


def build_nc(n_tok: int = N_TOK_FULL, collectives: bool = True):
    """Build the SPMD program (identical on all 8 cores)."""
    NCH = n_tok // 512   # 512-token chunks
    KT = n_tok // 128    # 128-token key tiles
    TOKT = n_tok // 128  # vaug token tiles
    WTOK = n_tok // 16   # tokens per core per batch per a2a wave
    QCW = max(NCH // 2, 1)  # q-chunks per proj wave

    nc = bacc.Bacc("TRN2", target_bir_lowering=False, debug=False, num_devices=8)

    xt_e = nc.dram_tensor("xt", [C, n_tok], BF16, kind="ExternalInput")
    wt_e = nc.dram_tensor("wt", [6, P, 8, P], BF16, kind="ExternalInput")
    wpt_e = nc.dram_tensor("wpt", [C, C], BF16, kind="ExternalInput")
    gcol_e = nc.dram_tensor("gcol", [P, 8], F32, kind="ExternalInput")
    lpec_e = nc.dram_tensor("lpec", [P, 4], F32, kind="ExternalInput")
    biasb_e = nc.dram_tensor("biasb", [P, C], F32, kind="ExternalInput")
    y_e = nc.dram_tensor("y", [2, 2, WTOK, C], F32, kind="ExternalOutput")

    xt_ap = xt_e.ap().rearrange("(o p) t -> p o t", p=P)       # [128, 8, n_tok]
    wpt_ap = wpt_e.ap().rearrange("(o p) c -> p o c", p=P)     # [128, 8, 1024]

    with tile.TileContext(nc) as tc:
        with (
            tc.tile_pool(name="const", bufs=1) as cpool,
            tc.tile_pool(name="persist", bufs=1) as ppool,
            tc.tile_pool(name="work", bufs=2) as wpool,
            tc.tile_pool(name="et", bufs=3) as etpool,
            tc.tile_pool(name="dram", bufs=1, space="DRAM") as dram,
        ):
            # ---- constants / persistent tiles ----
            wt_sb = ppool.tile([P, 6, 8, P], BF16)
            for f in range(6):
                nc.scalar.dma_start(wt_sb[:, f], wt_e.ap()[f])
            xall = ppool.tile([P, NCH, 8, 512], BF16)
            for c in range(NCH):
                nc.sync.dma_start(xall[:, c], xt_ap[:, :, ts(c, 512)])

            gcols = cpool.tile([P, 8], F32)
            lpecs = cpool.tile([P, 4], F32)
            with tc.high_priority(offset=-1000):
                nc.scalar.dma_start(gcols[:], gcol_e[:])
                nc.scalar.dma_start(lpecs[:], lpec_e[:])
            scol = cpool.tile([P, 1], BF16)
            nc.any.memset(scol[:], 1.0 / C)
            ones1 = cpool.tile([1, P], F32R)
            nc.any.memset(ones1[:].bitcast(F32), 1.0)
            ident = cpool.tile([P, P], F32R)
            make_identity(nc, ident[:].bitcast(F32))

            qf = ppool.tile([P, 2, n_tok], BF16)   # q feature-major [f, fo, tok]
            kf = ppool.tile([P, 2, n_tok], BF16)
            vaug = ppool.tile([P, TOKT, 4, D + 1], F32R)  # [tok, to, head, d|1]
            resid = ppool.tile([P, 2, n_tok], BF16)       # v_lpe + attn out
            nc.gpsimd.memset(vaug[:, :, :, D : D + 1].bitcast(F32), 1.0)

            # proj weights / bias tiles (loaded after P1-B: see below)
            wpt_sb = ppool.tile([P, 8, C], BF16)
            biasb_sb = ppool.tile([P, C], F32)

            # stats scratch (per chunk)
            st_in = [dram.tile([4, 512], BF16, name=f"stin{c}") for c in range(NCH)]
            st_out = [dram.tile([4, 512], BF16, name=f"stout{c}") for c in range(NCH)]
            rows_d = [dram.tile([2, 1, 2, 512], BF16, name=f"rows{c}") for c in range(NCH)]
            a2a_in = [dram.tile([8, LOCF, WTOK], BF16, name=f"a2ai{g}") for g in range(2)]
            a2a_out = [dram.tile([8, LOCF, WTOK], BF16, name=f"a2ao{g}") for g in range(2)]

            rbs = {}  # (chunk, qk2) -> broadcast tile [P, 2(rs|mrs), 512]

            def emit_rows(c):
                """Stats post-processing for chunk c: load reduced stats,
                derive rstd & mu*rstd rows, ship to DRAM, broadcast."""
                mu2 = wpool.tile([1, 2, 512], BF16, tag="mu2", bufs=1)
                s22 = wpool.tile([1, 2, 512], BF16, tag="s22", bufs=1)
                stv = st_out[c].rearrange("(q k) t -> k q t", k=2)
                nc.sync.dma_start(mu2[:], stv[0])   # rows (mu_q, mu_k)
                nc.sync.dma_start(s22[:], stv[1])   # rows (s2_q, s2_k)
                t2 = wpool.tile([1, 2, 512], BF16, tag="t2", bufs=1)
                nc.vector.tensor_tensor(t2[:], mu2[:], mu2[:], ALU.mult)
                nc.vector.tensor_tensor(t2[:], s22[:], t2[:], ALU.subtract)
                t2e = wpool.tile([1, 2, 512], BF16, tag="t2e", bufs=1)
                nc.vector.tensor_scalar(t2e[:], t2[:], 1.0, 1e-5, ALU.mult, ALU.add)
                rv = wpool.tile([1, 2, 512], BF16, tag="rv", bufs=1)
                with nc.allow_low_precision(reason="LN rstd in bf16"):
                    nc.vector.reciprocal(rv[:], t2e[:])
                rs2 = wpool.tile([1, 2, 512], BF16, tag="rs2", bufs=1)
                nc.scalar.activation(rs2[:], rv[:], AF.Sqrt)
                mrs2 = wpool.tile([1, 2, 512], BF16, tag="mrs2", bufs=1)
                nc.vector.tensor_tensor(mrs2[:], mu2[:], rs2[:], ALU.mult)
                if c == NCH - 1:
                    # flip the Act table to the exp set before attention
                    warm = wpool.tile([1, 16], F32R, tag="warm", bufs=1)
                    nc.scalar.activation(warm[:], mrs2[:, 0, 0:16], AF.Exp)
                rdv = rows_d[c]  # [2(qk), 1, 2(kind), 512]
                nc.scalar.dma_start(rdv[:, 0, 0, :], rs2[:])
                nc.scalar.dma_start(rdv[:, 0, 1, :], mrs2[:])
                for qk2 in range(2):
                    rb = wpool.tile([P, 2, 512], BF16, tag="rb", bufs=4)
                    nc.scalar.dma_start(rb[:], rdv[qk2].to_broadcast([P, 2, 512]))
                    rbs[(c, qk2)] = rb

            def emit_norm(c):
                """Apply LN to q,k for chunk c (k first: attention needs k of
                every chunk before q-chunk 0's last key tile)."""
                for qk2, feat in ((1, kf), (0, qf)):
                    rb = rbs[(c, qk2)]
                    t1 = wpool.tile([P, 2, 512], BF16, tag="t1")
                    nc.vector.tensor_tensor(
                        t1[:], feat[:, :, ts(c, 512)],
                        rb[:, 0:1, :].to_broadcast([P, 2, 512]), ALU.mult,
                    )
                    nc.vector.tensor_tensor(
                        t1[:], t1[:],
                        rb[:, 1:2, :].to_broadcast([P, 2, 512]), ALU.subtract,
                    )
                    for fo in range(2):
                        nc.vector.tensor_scalar(
                            feat[:, fo, ts(c, 512)], t1[:, fo, :],
                            gcols[:, 4 * qk2 + fo : 4 * qk2 + fo + 1],
                            gcols[:, 4 * qk2 + 2 + fo : 4 * qk2 + 3 + fo],
                            ALU.mult, ALU.add,
                        )

            # ============ P1-A: q,k projection + stats (per chunk) ============
            # chunk order: the last-processed chunk's LN stats arrive last;
            # make that chunk 2 (its keys are consumed 7us into attention,
            # its queries only at iteration 4) instead of chunk 3.
            CH = list(range(NCH))
            if NCH >= 2:
                CH[-1], CH[-2] = CH[-2], CH[-1]
            with tc.tile_pool(name="psP1", bufs=1, space="PSUM") as psP1:
                for ci, c in enumerate(CH):
                    for fo4 in range(4):
                        ps = psP1.tile([P, 512], F32, tag="mm", bufs=3)
                        for co in range(8):
                            nc.tensor.matmul(
                                ps[:], wt_sb[:, fo4, co, :], xall[:, c, co, :],
                                start=(co == 0), stop=(co == 7),
                            )
                        dest = qf if fo4 < 2 else kf
                        nc.scalar.copy(dest[:, fo4 % 2, ts(c, 512)], ps[:])
                    sqq = wpool.tile([P, 2, 512], BF16, tag="sq")
                    nc.vector.tensor_tensor(
                        sqq[:], qf[:, :, ts(c, 512)], qf[:, :, ts(c, 512)], ALU.mult
                    )
                    sqk = wpool.tile([P, 2, 512], BF16, tag="sq")
                    nc.vector.tensor_tensor(
                        sqk[:], kf[:, :, ts(c, 512)], kf[:, :, ts(c, 512)], ALU.mult
                    )
                    strow = wpool.tile([1, 4, 512], BF16, tag="strow", bufs=1)
                    # order: S1q, S1k (no sq dep) then S2q, S2k
                    stats = [
                        (0, qf[:, :, ts(c, 512)]),
                        (2, kf[:, :, ts(c, 512)]),
                        (1, sqq[:]),
                        (3, sqk[:]),
                    ]
                    for row, src in stats:
                        pst = psP1.tile([1, 512], F32, tag="stat", bufs=2)
                        for fo in range(2):
                            nc.tensor.matmul(
                                pst[:], scol[:], src[:, fo, :],
                                start=(fo == 0), stop=(fo == 1),
                            )
                        nc.gpsimd.tensor_copy(strow[:, row, :], pst[:])
                    nc.sync.dma_start(st_in[c][:], strow[:])
                    if collectives:
                        nc.gpsimd.collective_compute(
                            "AllReduce", ALU.add, replica_groups=GROUPS,
                            ins=[st_in[c].opt()], outs=[st_out[c].opt()],
                        )
                    else:
                        nc.gpsimd.dma_start(st_out[c][:], st_in[c][:])
                    if ci >= 1:
                        emit_rows(CH[ci - 1])
                    if ci >= 2:
                        emit_norm(CH[ci - 2])

                # ============ P1-B: v (feature-major) + LPE + transpose ======
                # transposes run one chunk behind the v matmuls so the PE
                # never waits on the PSUM->SBUF vtmp hop; the last chunk's
                # LN application is emitted after its vtmp copies so the DVE
                # queue is not blocked on the (late) chunk-NCH-1 broadcast.
                tasks = [lambda: emit_rows(CH[-1])]
                if NCH >= 2:
                    tasks += [lambda: emit_norm(CH[-2])]
                def emit_tr(c, vtmps):
                    for tj in range(4):
                        to = 4 * c + tj
                        ptr = psP1.tile([P, 256], F32R, tag="tr", bufs=2)
                        for fo in range(2):
                            nc.tensor.transpose(
                                ptr[:, ts(fo, P)], vtmps[fo][:, ts(tj, P)], ident[:]
                            )
                        pr = ptr[:].rearrange("p (g d) -> p g d", g=4)
                        nc.gpsimd.tensor_copy(vaug[:, to, :, 0:D], pr[:])
                prev_vt = None
                for c in range(NCH):
                    vtmps = []
                    for fo in range(2):
                        psv = psP1.tile([P, 512], F32, tag="mm", bufs=3)
                        for co in range(8):
                            nc.tensor.matmul(
                                psv[:], wt_sb[:, 4 + fo, co, :], xall[:, c, co, :],
                                start=(co == 0), stop=(co == 7),
                            )
                        vtmp = wpool.tile([P, 512], F32R, tag="vtmp", bufs=4)
                        nc.vector.tensor_copy(vtmp[:].bitcast(F32), psv[:])
                        nc.scalar.activation(
                            resid[:, fo, ts(c, 512)], psv[:], AF.Identity,
                            scale=lpecs[:, fo : fo + 1], bias=lpecs[:, 2 + fo : 3 + fo],
                        )
                        vtmps.append(vtmp)
                    if prev_vt is not None:
                        emit_tr(c - 1, prev_vt)
                    prev_vt = vtmps
                    if tasks:
                        tasks.pop(0)()
                emit_norm(CH[-1])
                emit_tr(NCH - 1, prev_vt)
                tasks = []
                with tc.high_priority(offset=-1000000):
                    for co in range(8):
                        nc.gpsimd.dma_start(wpt_sb[:, co, :], wpt_ap[:, co, :])
                    nc.gpsimd.dma_start(biasb_sb[:], biasb_e[:])

            # ============ P6+P8: attention + projection waves ============
            with tc.tile_pool(name="psATT", bufs=1, space="PSUM") as psA2:

                pending = [None]
                pend_avs = []
                pend_rc = [None]

                def emit_recips():
                    if pending[0] is None:
                        return
                    psav0, psav1, qc, fo = pending[0]
                    rc = wpool.tile([1, 2, 512], F32R, tag="rc")
                    with nc.allow_low_precision(reason="softmax 1/sumexp as fp32r"):
                        nc.vector.reciprocal(rc[:, 0, :], psav0[D : D + 1, :])
                        nc.vector.reciprocal(rc[:, 1, :], psav1[D : D + 1, :])
                    pend_rc[0] = rc

                def emit_epilogue():
                    if pending[0] is None:
                        return
                    if pend_rc[0] is None:
                        emit_recips()
                    psav0, psav1, qc, fo = pending[0]
                    pending[0] = None
                    rc = pend_rc[0]
                    pend_rc[0] = None
                    prc0 = psA2.tile([P, 512], F32, tag="sc", bufs=6)
                    prc1 = psA2.tile([P, 512], F32, tag="sc", bufs=6)
                    nc.tensor.matmul(prc0[:], ones1[:], rc[:, 0, :], start=True, stop=True)
                    nc.tensor.matmul(prc1[:], ones1[:], rc[:, 1, :], start=True, stop=True)
                    t_c = wpool.tile([P, 512], F32R, tag="tc")
                    nc.vector.tensor_copy(t_c[64:128, :].bitcast(F32), psav1[0:D, :])
                    t_o = wpool.tile([P, 512], F32R, tag="to")
                    nc.vector.tensor_tensor(
                        t_o[0:D, :], psav0[0:D, :], prc0[0:D, :], ALU.mult
                    )
                    nc.vector.tensor_tensor(
                        t_o[64:128, :], t_c[64:128, :], prc1[64:128, :], ALU.mult
                    )
                    nc.vector.tensor_tensor(
                        resid[0:D, fo, ts(qc, 512)],
                        resid[0:D, fo, ts(qc, 512)], t_o[0:D, :], ALU.add,
                    )
                    nc.vector.tensor_tensor(
                        resid[64:128, fo, ts(qc, 512)],
                        resid[64:128, fo, ts(qc, 512)], t_o[64:128, :], ALU.add,
                    )

                def emit_proj_wave(g):
                    pjb = wpool.tile([P, 2, 8, WTOK], BF16, tag="pjb", bufs=2)
                    for bp in range(2):
                        eng_d = nc.sync if bp == 0 else nc.scalar
                        eng_d.dma_start(
                            pjb[:, bp],
                            a2a_out[g][4 * bp : 4 * bp + 4].rearrange(
                                "r (ci p) t -> p (r ci) t", p=P
                            ),
                        )
                    for bp in range(2):
                        for nch2 in range(2):
                            psy = psA2.tile([P, 512], F32, tag="av", bufs=2)
                            for jc in range(8):
                                nc.tensor.matmul(
                                    psy[0:WTOK, :], pjb[:, bp, jc, :],
                                    wpt_sb[:, jc, ts(nch2, 512)],
                                    start=(jc == 0), stop=(jc == 7),
                                )
                            yt = wpool.tile([P, 512], F32, tag="yt")
                            nc.vector.tensor_tensor(
                                yt[0:WTOK, :], psy[0:WTOK, :],
                                biasb_sb[0:WTOK, ts(nch2, 512)], ALU.add,
                            )
                            eng_y = nc.sync if nch2 == 0 else nc.scalar
                            eng_y.dma_start(
                                y_e[g, bp, :, ts(nch2, 512)], yt[0:WTOK, :]
                            )

                for i in range(2 * NCH):
                    qc, fo = i // 2, i % 2
                    psav0 = psA2.tile([P, 512], F32, tag="av", bufs=2)
                    psav1 = psA2.tile([P, 512], F32, tag="av", bufs=2)
                    ets = {}
                    if pend_avs:
                        pass  # emitted inside the kt loop below
                    for kt in range(KT):
                        et = etpool.tile([P, 2, 512], F32R, tag="et")
                        psc0 = psA2.tile([P, 512], F32, tag="sc", bufs=6)
                        nc.tensor.matmul(
                            psc0[:], kf[0:64, fo, ts(kt, P)],
                            qf[0:64, fo, ts(qc, 512)], start=True, stop=True,
                        )
                        nc.scalar.activation(et[:, 0, :], psc0[:], AF.Exp)
                        psc1 = psA2.tile([P, 512], F32, tag="sc", bufs=6)
                        nc.tensor.matmul(
                            psc1[:], kf[64:128, fo, ts(kt, P)],
                            qf[64:128, fo, ts(qc, 512)], start=True, stop=True,
                        )
                        eng = nc.gpsimd if kt < KT // 2 else nc.vector
                        eng.tensor_scalar(
                            et[:, 1, :].bitcast(I32), psc1[:],
                            EXP_A, EXP_B, ALU.mult, ALU.add,
                        )
                        ets[kt] = et
                        if kt < 2 and pend_avs:
                            pend_avs.pop(0)()
                        if kt == 2:
                            emit_recips()
                        if kt == 4:
                            emit_epilogue()
                        if kt >= 2:
                            _av(nc, vaug, psav0, psav1, fo, kt - 2, ets.pop(kt - 2), KT)
                    for kt in (KT - 2, KT - 1):
                        pend_avs.append(
                            lambda kt=kt, p0=psav0, p1=psav1, f=fo, e=ets.pop(kt): _av(
                                nc, vaug, p0, p1, f, kt, e, KT
                            )
                        )
                    pending[0] = (psav0, psav1, qc, fo)

                    # end of a proj wave's q-chunks: flush epilogue, ship shards
                    if fo == 1 and (qc + 1) % QCW == 0:
                        g = qc // QCW
                        for f_ in pend_avs:
                            f_()
                        pend_avs = []
                        emit_epilogue()
                        base = g * (n_tok // 2)
                        for j in range(8):
                            eng_d = nc.sync if j % 2 == 0 else nc.scalar
                            eng_d.dma_start(
                                a2a_in[g][j].rearrange("(f p) t -> p f t", p=P),
                                resid[:, :, base + j * WTOK : base + (j + 1) * WTOK],
                            )
                        if collectives:
                            nc.gpsimd.collective_compute(
                                "AllToAll", ALU.bypass,
                                replica_groups=[list(range(8))],
                                ins=[a2a_in[g].opt()], outs=[a2a_out[g].opt()],
                            )
                        else:
                            nc.gpsimd.dma_start(a2a_out[g][:], a2a_in[g][:])
                emit_epilogue()
                emit_proj_wave(0)
                pwarm = psA2.tile([P, 512], F32, tag="sc", bufs=6)
                for _ in range(40):
                    nc.tensor.matmul(
                        pwarm[:], kf[0:64, 0, 0:P], qf[0:64, 0, 0:512],
                        start=True, stop=True,
                    )
                emit_proj_wave(1)

    nc.compile()
    return nc


def _av(nc, vaug, psav0, psav1, fo, kt, et, KT):
    """AV accumulation for key tile kt of head pair fo."""
    nc.tensor.matmul(
        psav0[0 : D + 1, :], vaug[:, kt, 2 * fo, :], et[:, 0, :],
        start=(kt == 0), stop=(kt == KT - 1),
    )
    nc.tensor.matmul(
        psav1[0 : D + 1, :], vaug[:, kt, 2 * fo + 1, :], et[:, 1, :],
        start=(kt == 0), stop=(kt == KT - 1),
    )


def prep_in_maps(
    x, w_qkv, q_gamma, q_beta, k_gamma, k_beta, lpe_w, lpe_b, w_proj, b_proj,
    n_tok: int = N_TOK_FULL,
):
    """Shard the full inputs into the 8 per-core input maps."""
    x = np.asarray(x, np.float32)
    w_qkv = np.asarray(w_qkv, np.float32)
    w_proj = np.asarray(w_proj, np.float32)
    vecs = [np.asarray(v, np.float32) for v in
            (q_gamma, q_beta, k_gamma, k_beta, lpe_w, lpe_b, b_proj)]
    q_gamma, q_beta, k_gamma, k_beta, lpe_w, lpe_b, b_proj = vecs

    scale = float(D) ** -0.5
    wq, wk, wv = w_qkv[0:C], w_qkv[C : 2 * C], w_qkv[2 * C : 3 * C]
    wpt = to_bf16(w_proj.T)
    biasb = np.ascontiguousarray(np.broadcast_to(b_proj, (P, C)), np.float32)

    in_maps = []
    for c in range(8):
        b_, hg = c // 4, c % 4
        sl = slice(LOCF * hg, LOCF * hg + LOCF)
        xt = to_bf16(x[b_, :n_tok].T)
        blocks = [wq[sl][0:P], wq[sl][P:LOCF], wk[sl][0:P], wk[sl][P:LOCF],
                  wv[sl][0:P], wv[sl][P:LOCF]]
        wt = np.stack([bl.T.reshape(8, P, P).transpose(1, 0, 2) for bl in blocks])
        wt = to_bf16(wt)

        def two(v):
            return v[sl].reshape(2, P).T  # [p, fo]

        gq = two(q_gamma) * scale
        bq = two(q_beta) * scale
        gk = two(k_gamma)
        bk = two(k_beta)
        gcol = np.concatenate([gq, bq, gk, bk], axis=1).astype(np.float32)  # [128,8]
        lpec = np.concatenate([two(lpe_w), two(lpe_b)], axis=1).astype(np.float32)
        in_maps.append(
            {
                "xt": xt,
                "wt": wt,
                "wpt": wpt,
                "gcol": np.ascontiguousarray(gcol),
                "lpec": np.ascontiguousarray(lpec),
                "biasb": biasb,
            }
        )
    return in_maps


def assemble_y(results, n_tok: int = N_TOK_FULL) -> np.ndarray:
    WTOK = n_tok // 16
    y = np.empty((B, n_tok, C), np.float32)
    for c in range(8):
        yc = np.asarray(results[c]["y"]).reshape(2, 2, WTOK, C)
        for g in range(2):
            lo = g * (n_tok // 2) + c * WTOK
            y[0, lo : lo + WTOK] = yc[g, 0]
            y[1, lo : lo + WTOK] = yc[g, 1]
    return y


_NC_CACHE = {}


def kernel(**inputs) -> np.ndarray:
    key = ("full", N_TOK_FULL)
    if key not in _NC_CACHE:
        _NC_CACHE[key] = build_nc(N_TOK_FULL, collectives=True)
    nc = _NC_CACHE[key]
    in_maps = prep_in_maps(**inputs)
    res = run_bass_kernel_spmd(nc, in_maps, core_ids=list(range(8)))
    return assemble_y(res.results)
